# revision 1
# baseline (speedup 1.0000x reference)
"""Trainium2 Bass kernel for nn_EquivariantDecoder (GNN message passing).

Sharding: nodes are split into 8 contiguous ranges of 6272 (= 49 tiles of
128); each core owns the edges whose dst lands in its range, so per-node
segment sums are core-local (no collectives). Edges are sorted by dst on
the host and padded so every (core, node-tile) group holds exactly K
tiles of 128 edge slots; the K is baked into the traced program.

Device work per core:
  edge path:  w = silu(m_ij @ W1 + b1) @ W2 + b2   (tile-transposed m_ij)
              scatter-sum of rel*w into the 128-node tile via a one-hot
              matmul (one-hot built on device from dst%128 with iota +
              is_equal; padding slots use col=-1 so they vanish)
  node path:  alpha = silu(h @ vgW1 + vgb1) @ vgW2 + vgb2
              out = sum_k alpha_k * vel_k + scatter_sum * (1/max(cnt,1))
"""

import sys

import numpy as np

try:
    import concourse.bass as bass  # noqa: F401
except Exception:  # pragma: no cover
    sys.path.insert(0, "/opt/trn_rl_repo")

import concourse.bass as bass
import concourse.mybir as mybir
from concourse.bass_utils import run_bass_kernel_spmd
from concourse.tile import TileContext
from concourse.vector_clock import ScopedClock

N_NODES = 50000
N_EDGES = 800000
H = 256
N_CORES = 8
NT = 49                 # node tiles per core
NPC = NT * 128          # 6272 nodes per core
N_PAD = N_CORES * NPC   # 50176
P = 128

# edge-MLP matmul dtype: bfloat16 | float32r | float32
EDGE_DT = mybir.dt.bfloat16
EDGE_NP = mybir.dt.np(EDGE_DT)
F32 = mybir.dt.float32
AF = mybir.ActivationFunctionType
OP = mybir.AluOpType


# ---------------------------------------------------------------------------
# Walrus on this toolchain rejects >2 sync waits on the TileContext tail
# drain ("Too many sync wait commands"); split them across SP NOPs.
def _patched_drain_and_barrier(self, tick_clock, wait_clock):
    drain_inst = self.nc.sync.drain()
    wait_clock.add_sem_waits(
        drain_inst.ins, ScopedClock({None: tick_clock.global_clock})
    )
    si = drain_inst.ins.sync_info
    if si is not None and si.on_wait and len(si.on_wait) > 1:
        extra = list(si.on_wait[1:])
        del si.on_wait[1:]
        for w in extra:
            nop = self.nc.sync.nop(nofuse=True, hint="drain_wait_split")
            nsi = nop.ins.sync_info
            if nsi is None:
                nop.ins.sync_info = mybir.SyncInfo(on_wait=[w], on_update=[])
            else:
                nsi.on_wait.append(w)

    self.nc.all_engine_barrier()
    assert self.sems is not None
    popped = self.nc._tile_sem_poison_stack.pop()
    assert popped is self._sem_poison
    self.nc.clear_and_free_semaphores(list(self.sems.allocated().values()))
    self.nc.all_engine_barrier()


TileContext._drain_and_barrier = _patched_drain_and_barrier


def _split_excess_waits(nc, maxw: int = 1):
    """Walrus rejects >maxw sync waits on one instruction; move the excess
    onto NOPs inserted just before, on the same engine (same-queue program
    order makes this equivalent)."""
    n_split = 0
    for f in nc.m.functions:
        for b in f.blocks:
            out = []
            for inst in b.instructions:
                si = inst.sync_info
                if si is not None and si.on_wait and len(si.on_wait) > maxw:
                    extra = list(si.on_wait[: -maxw])
                    del si.on_wait[: -maxw]
                    for i in range(0, len(extra), maxw):
                        nop = mybir.InstNoOp(
                            name=f"{inst.name}-wsplit{i}",
                            engine=inst.engine,
                            sync_info=mybir.SyncInfo(
                                on_wait=extra[i:i + maxw], on_update=[]),
                            bass_nofuse=True,
                        )
                        out.append(nop)
                    n_split += 1
                out.append(inst)
            b.instructions[:] = out
    return n_split
# ---------------------------------------------------------------------------


def _build_program(K: int, b2: float):
    """Trace the single-core SPMD program for a fixed K (edge tiles per
    node-tile group)."""
    ET = NT * K                      # edge tiles per core
    n_mac = (ET + 7) // 8            # macros of up to 8 edge tiles
    n_sup = (n_mac + 1) // 2         # supertiles of 2 macros (1 DMA each)

    nc = bass.Bass()

    mijT = nc.dram_tensor("mijT", [n_sup, P, 4096], EDGE_DT, kind="ExternalInput")
    relw_d = nc.dram_tensor("relw", [P, ET * 4], F32, kind="ExternalInput")
    hT = nc.dram_tensor("hT", [NT, P, 2 * P], EDGE_DT, kind="ExternalInput")
    velg_d = nc.dram_tensor("velg", [P, NT * 16], F32, kind="ExternalInput")
    w1b_d = nc.dram_tensor("w1b", [2, 2, P, P], EDGE_DT, kind="ExternalInput")
    w2t_d = nc.dram_tensor("w2t", [2, P, 1], EDGE_DT, kind="ExternalInput")
    b1t_d = nc.dram_tensor("b1t", [2, P, 1], F32, kind="ExternalInput")
    vgw1b_d = nc.dram_tensor("vgw1b", [2, 2, P, P], EDGE_DT, kind="ExternalInput")
    vgw2t_d = nc.dram_tensor("vgw2t", [2, P, 5], EDGE_DT, kind="ExternalInput")
    vgb1t_d = nc.dram_tensor("vgb1t", [2, P, 1], F32, kind="ExternalInput")
    onesb2_d = nc.dram_tensor("onesb2", [1, 133], EDGE_DT, kind="ExternalInput")
    out_d = nc.dram_tensor("out", [P, NT * 3], F32, kind="ExternalOutput")

    with TileContext(nc) as tc:
        with (
            tc.tile_pool(name="const", bufs=1) as cpool,
            tc.tile_pool(name="rhs", bufs=3) as rhs_pool,
            tc.tile_pool(name="s1", bufs=3) as s1_pool,
            tc.tile_pool(name="small", bufs=6) as sm_pool,
            tc.tile_pool(name="oh", bufs=4) as oh_pool,
            tc.tile_pool(name="nodes", bufs=3) as nd_pool,
            tc.tile_pool(name="ps_mm1", bufs=2, space="PSUM") as ps1_pool,
            tc.tile_pool(name="ps_w", bufs=2, space="PSUM") as psw_pool,
            tc.tile_pool(name="ps_sc", bufs=2, space="PSUM") as pssc_pool,
        ):
            # ---- constants ----
            w1 = [[cpool.tile([P, P], EDGE_DT, tag=f"w1_{kk}{hh}", name=f"w1_{kk}{hh}")
                   for hh in range(2)] for kk in range(2)]
            vgw1 = [[cpool.tile([P, P], EDGE_DT, tag=f"vgw1_{kk}{hh}", name=f"vgw1_{kk}{hh}")
                     for hh in range(2)] for kk in range(2)]
            for kk in range(2):
                for hh in range(2):
                    nc.sync.dma_start(w1[kk][hh][:], w1b_d[kk, hh, :, :])
                    nc.sync.dma_start(vgw1[kk][hh][:], vgw1b_d[kk, hh, :, :])
            w2 = [cpool.tile([P, 1], EDGE_DT, tag=f"w2_{hh}", name=f"w2_{hh}") for hh in range(2)]
            b1 = [cpool.tile([P, 1], F32, tag=f"b1_{hh}", name=f"b1_{hh}") for hh in range(2)]
            vgw2 = [cpool.tile([P, 5], EDGE_DT, tag=f"vgw2_{hh}", name=f"vgw2_{hh}") for hh in range(2)]
            vgb1 = [cpool.tile([P, 1], F32, tag=f"vgb1_{hh}", name=f"vgb1_{hh}") for hh in range(2)]
            for hh in range(2):
                nc.sync.dma_start(w2[hh][:], w2t_d[hh, :, :])
                nc.sync.dma_start(b1[hh][:], b1t_d[hh, :, :])
                nc.sync.dma_start(vgw2[hh][:], vgw2t_d[hh, :, :])
                nc.sync.dma_start(vgb1[hh][:], vgb1t_d[hh, :, :])
            onesb2 = cpool.tile([1, 133], EDGE_DT, tag="onesb2")
            nc.sync.dma_start(onesb2[:], onesb2_d[0, :][None, :])

            # iota values 0..127 repeated 8x: one-hot build for a whole macro
            iota = cpool.tile([P, 8 * P], EDGE_DT, tag="iota")
            nc.gpsimd.iota(iota[:], pattern=[[0, 8], [1, P]], base=0,
                           channel_multiplier=0,
                           allow_small_or_imprecise_dtypes=True)

            relw = cpool.tile([P, ET * 4], F32, tag="relw")
            nc.scalar.dma_start(relw[:], relw_d[:, :])
            velg = cpool.tile([P, NT * 16], F32, tag="velg")
            nc.scalar.dma_start(velg[:], velg_d[:, :])

            # packed output, one column triple per node tile; single DMA at end
            outbuf = cpool.tile([P, NT * 3], F32, tag="outbuf")

            # node-tail emission, called when node tile nt's scatter psum is
            # fully accumulated
            def node_tail(nt: int, ps_sc):
                geom = sm_pool.tile([P, 3], F32, tag="geom")
                nc.vector.tensor_scalar(
                    geom[:], ps_sc[:, 0:3], velg[:, nt * 16 + 15:nt * 16 + 16],
                    None, op0=OP.mult)

                rhn = nd_pool.tile([P, 2 * P], EDGE_DT, tag="rhn")
                eng = nc.sync if nt % 2 == 0 else nc.scalar
                eng.dma_start(rhn[:], hT[nt, :, :])
                psn = [ps1_pool.tile([P, P], F32, tag="ps_mm1", name="psn")
                       for _ in range(2)]
                for hh in range(2):
                    for kk in range(2):
                        nc.tensor.matmul(psn[hh][:], vgw1[kk][hh][:],
                                         rhn[:, kk * P:(kk + 1) * P],
                                         start=(kk == 0), stop=(kk == 1))
                s1n = [nd_pool.tile([P, P], EDGE_DT, tag=f"s1n_{hh}", name=f"s1n_{hh}")
                       for hh in range(2)]
                for hh in range(2):
                    nc.scalar.activation(s1n[hh][:], psn[hh][:], AF.Silu,
                                         bias=vgb1[hh][:, 0:1], scale=1.0)
                psa = psw_pool.tile([P, 8], F32, tag="ps_w")
                for hh in range(2):
                    nc.tensor.matmul(psa[:, 0:5], s1n[hh][:], vgw2[hh][:],
                                     start=(hh == 0), stop=False)
                nc.tensor.matmul(psa[:, 0:5], onesb2[:, 0:128],
                                 onesb2[:, 128:133], start=False, stop=True)

                # out[:, j] = geom[:, j] + sum_k alpha[k] * vel[j, k]
                scratch = sm_pool.tile([P, 15], F32, tag="scratch")
                vbase = nt * 16
                velg_v = velg[:, vbase:vbase + 15].rearrange(
                    "p (j k) -> p j k", k=5)
                nc.vector.tensor_tensor(
                    scratch[:].rearrange("p (j k) -> p j k", k=5),
                    velg_v,
                    psa[:, None, 0:5].broadcast_to([P, 3, 5]),
                    op=OP.mult)
                acc = sm_pool.tile([P, 3], F32, tag="acc")
                nc.vector.tensor_reduce(
                    acc[:, :, None],
                    scratch[:].rearrange("p (j k) -> p j k", k=5),
                    axis=mybir.AxisListType.X, op=OP.add)
                nc.vector.tensor_add(outbuf[:, nt * 3:nt * 3 + 3],
                                     acc[:], geom[:])

            # ---- edge-path macro loop (8 edge tiles per macro) ----
            ps_sc = None
            sup = None
            for m in range(n_mac):
                t0 = m * 8
                G = min(8, ET - t0)          # real edge tiles in this macro
                W = G * P                    # macro width in edges
                si, sm = divmod(m, 2)
                if sm == 0:
                    sup = rhs_pool.tile([P, 4096], EDGE_DT, tag="sup",
                                        name="sup")
                    eng = nc.sync if si % 2 == 0 else nc.scalar
                    eng.dma_start(sup[:], mijT[si, :, :])
                rhs = sup[:, sm * 2048:(sm + 1) * 2048]
                ps1 = [ps1_pool.tile([P, 1024], F32, tag="ps_mm1", name="ps1")
                       for _ in range(2)]
                n_ch = (W + 511) // 512
                for hh in range(2):
                    for ch in range(n_ch):
                        cw = min(512, W - ch * 512)
                        for kk in range(2):
                            nc.tensor.matmul(
                                ps1[hh][:, ch * 512:ch * 512 + cw],
                                w1[kk][hh][:],
                                rhs[:, kk * 1024 + ch * 512:
                                    kk * 1024 + ch * 512 + cw],
                                start=(kk == 0), stop=(kk == 1))
                s1 = [s1_pool.tile([P, 1024], EDGE_DT, tag=f"s1_{hh}", name=f"s1_{hh}")
                      for hh in range(2)]
                for hh in range(2):
                    nc.scalar.activation(s1[hh][:, 0:W], ps1[hh][:, 0:W],
                                         AF.Silu,
                                         bias=b1[hh][:, 0:1], scale=1.0)
                psw = psw_pool.tile([P, 8], F32, tag="ps_w")
                for c in range(G):
                    for hh in range(2):
                        nc.tensor.matmul(psw[:, c:c + 1],
                                         s1[hh][:, c * P:(c + 1) * P],
                                         w2[hh][:],
                                         start=(hh == 0), stop=(hh == 1))

                # msg for all chunks in one op: [P, G, 3] = (w + b2) * rel
                relw_v = relw.rearrange("p (t f) -> p t f", f=4)
                msg = sm_pool.tile([P, 24], EDGE_DT, tag="msg")
                nc.vector.scalar_tensor_tensor(
                    msg[:, 0:3 * G].rearrange("p (c f) -> p c f", f=3),
                    psw[:, 0:G, None].broadcast_to([P, G, 3]),
                    float(b2),
                    relw_v[:, t0:t0 + G, 0:3],
                    op0=OP.add, op1=OP.mult)
                # one-hot for all chunks in one op: [P, G*128]
                oh = oh_pool.tile([P, 8 * P], EDGE_DT, tag="oh")
                nc.vector.tensor_tensor(
                    oh[:, 0:W].rearrange("p (c f) -> p c f", f=P),
                    iota[:, 0:W].rearrange("p (c f) -> p c f", f=P),
                    relw_v[:, t0:t0 + G, 3:4].broadcast_to([P, G, P]),
                    op=OP.is_equal)

                for c in range(G):
                    t = t0 + c
                    nt, j = divmod(t, K)
                    if j == 0:
                        ps_sc = pssc_pool.tile([P, 3], F32, tag="ps_sc")
                    nc.tensor.matmul(ps_sc[:], oh[:, c * P:(c + 1) * P],
                                     msg[:, 3 * c:3 * c + 3],
                                     start=(j == 0), stop=(j == K - 1))
                    if j == K - 1:
                        node_tail(nt, ps_sc)

            nc.sync.dma_start(out_d[:, :], outbuf[:])

    _split_excess_waits(nc)
    return nc


def _preprocess(inputs: dict):
    """Shard + lay out all per-core device inputs. Returns (in_maps, K, b2)."""
    h = np.asarray(inputs["h"], np.float32)
    m_ij = np.asarray(inputs["m_ij"], np.float32)
    x = np.asarray(inputs["x"], np.float32)
    vel_all = np.asarray(inputs["vel_all"], np.float32)
    ei = np.asarray(inputs["edge_index"])
    src = ei[0].astype(np.int64)
    dst = ei[1].astype(np.int64)

    counts = np.bincount(dst, minlength=N_NODES).astype(np.float32)
    invc = (1.0 / np.maximum(counts, 1.0)).astype(np.float32)

    order = np.argsort(dst, kind="stable")
    dst_s = dst[order]
    src_s = src[order]
    g = dst_s // P                       # global 128-node group, 0..391
    n_groups = N_PAD // P                # 392
    cg = np.bincount(g, minlength=n_groups)
    K = max(1, int(-(-cg.max() // P)))   # ceil(max group)/128
    ET = NT * K
    n_mac = (ET + 3) // 4
    slots_core = ET * P

    gstart = np.zeros(n_groups, np.int64)
    gstart[1:] = np.cumsum(cg)[:-1]
    within = np.arange(N_EDGES, dtype=np.int64) - gstart[g]
    slot = g * (K * P) + within          # slot in global [392, K*128] layout

    Sg = n_groups * K * P
    colidx = np.full(Sg, -1.0, np.float32)
    colidx[slot] = (dst_s % P).astype(np.float32)
    relp = np.zeros((Sg, 3), np.float32)
    relp[slot] = x[src_s] - x[dst_s]
    mijp = np.zeros((Sg, H), EDGE_NP)
    mijp[slot] = m_ij[order].astype(EDGE_NP)

    # padded node tensors
    hp = np.zeros((N_PAD, H), np.float32)
    hp[:N_NODES] = h
    velp = np.zeros((N_PAD, 5, 3), np.float32)
    velp[:N_NODES] = vel_all
    invp = np.ones(N_PAD, np.float32)
    invp[:N_NODES] = invc

    # weights (shared by all cores)
    w1 = np.asarray(inputs["ew_W1"], np.float32)
    b1 = np.asarray(inputs["ew_b1"], np.float32)
    w2 = np.asarray(inputs["ew_W2"], np.float32)
    b2 = float(np.asarray(inputs["ew_b2"], np.float32)[0])
    vgw1 = np.asarray(inputs["vg_W1"], np.float32)
    vgb1 = np.asarray(inputs["vg_b1"], np.float32)
    vgw2 = np.asarray(inputs["vg_W2"], np.float32)
    vgb2 = np.asarray(inputs["vg_b2"], np.float32)

    w1b = w1.reshape(2, P, 2, P).transpose(0, 2, 1, 3).astype(EDGE_NP).copy()
    w2t = w2.reshape(2, P, 1).astype(EDGE_NP).copy()
    b1t = b1.reshape(2, P, 1).copy()
    vgw1b = vgw1.reshape(2, P, 2, P).transpose(0, 2, 1, 3).astype(EDGE_NP).copy()
    vgw2t = vgw2.reshape(2, P, 5).astype(EDGE_NP).copy()
    vgb1t = vgb1.reshape(2, P, 1).copy()
    onesb2 = np.zeros((1, 133), EDGE_NP)
    onesb2[0, :P] = 1.0
    onesb2[0, P:P + 5] = vgb2.astype(EDGE_NP)

    mijp = mijp.reshape(N_CORES, ET, P, H)
    relp = relp.reshape(N_CORES, ET, P, 3)
    colidx = colidx.reshape(N_CORES, ET, P)

    n_mac = (ET + 7) // 8
    n_sup = (n_mac + 1) // 2
    in_maps = []
    for k in range(N_CORES):
        # mijT supertiles: [n_sup, 128, 4096]; col = mac*1024 + kk*512
        # + chunk*128 + e, partition = h within kk half
        b = mijp[k].transpose(0, 2, 1).reshape(ET, 2, P, P)
        full = np.zeros((n_sup * 16, 2, P, P), EDGE_NP)
        full[:ET] = b
        mijT = np.ascontiguousarray(
            full.reshape(n_sup, 2, 8, 2, P, P).transpose(0, 4, 1, 3, 2, 5)
        ).reshape(n_sup, P, 4096)

        rw = np.empty((P, ET, 4), np.float32)
        rw[:, :, 0:3] = relp[k].transpose(1, 0, 2)
        rw[:, :, 3] = colidx[k].T
        relw = np.ascontiguousarray(rw).reshape(P, ET * 4)

        hk = hp[k * NPC:(k + 1) * NPC].reshape(NT, P, H)
        hTk = np.ascontiguousarray(
            hk.transpose(0, 2, 1).reshape(NT, 2, P, P).transpose(0, 2, 1, 3)
        ).reshape(NT, P, 2 * P).astype(EDGE_NP)

        # velg cols per node tile: [comp j, gate k] at 5*j+k, inv_count at 15
        vg = np.empty((P, NT, 16), np.float32)
        vg[:, :, 0:15] = (velp[k * NPC:(k + 1) * NPC]
                          .reshape(NT, P, 5, 3).transpose(1, 0, 3, 2)
                          .reshape(P, NT, 15))
        vg[:, :, 15] = invp[k * NPC:(k + 1) * NPC].reshape(NT, P).T
        velg = np.ascontiguousarray(vg).reshape(P, NT * 16)

        in_maps.append({
            "mijT": mijT,
            "relw": relw,
            "hT": hTk,
            "velg": velg,
            "w1b": w1b,
            "w2t": w2t,
            "b1t": b1t,
            "vgw1b": vgw1b,
            "vgw2t": vgw2t,
            "vgb1t": vgb1t,
            "onesb2": onesb2,
        })
    return in_maps, K, b2


def unpack_out(arr: np.ndarray) -> np.ndarray:
    """[128, NT*3] packed per-core output -> [NPC, 3]."""
    return arr.reshape(P, NT, 3).transpose(1, 0, 2).reshape(NPC, 3)


def kernel(**inputs) -> np.ndarray:
    in_maps, K, b2 = _preprocess(inputs)
    nc = _build_program(K, b2)
    res = run_bass_kernel_spmd(nc, in_maps, list(range(N_CORES)))
    parts = [unpack_out(res.results[k]["out"]) for k in range(N_CORES)]
    return np.concatenate(parts, axis=0)[:N_NODES].astype(np.float32)



# revision 3
# speedup vs baseline: 1.1171x; 1.1171x over previous
"""Trainium2 Bass kernel for nn_EquivariantDecoder (GNN message passing).

Sharding: nodes are split into 8 contiguous ranges of 6272 (= 49 tiles of
128); each core owns the edges whose dst lands in its range, so per-node
segment sums are core-local (no collectives). Edges are sorted by dst on
the host and padded so every (core, node-tile) group holds exactly K
tiles of 128 edge slots; the K is baked into the traced program.

v2 layout (fp8 edge path):
  mm1:  z = W1.T @ m_ij as ONE fp8 DoubleRow matmul per 512-edge chunk
        (K=256 contracted in a single pass, weights pre-scaled x16 so
        fp8 stays in normal range; un-scaled via the silu's input scale)
  silu: ACT, out in fp8 (s1), scale=1/16 folds the weight prescale
  mm2:  w = s1.T @ W2 per edge tile, fp8 stationary (fast weight load)
  scatter: one-hot matmul per edge tile (oh in fp8, msg in bf16);
        1/count, x16 of W2, and the b2*rel term are all folded on host:
        msg = psw * rel' with rel' = rel * invc[dst] / 256
  node: bf16 MLP as before; vg_b2 folded into a host-computed constant
        (velb2 + b2-geom term) added at the end.
"""

import sys

import numpy as np

try:
    import concourse.bass as bass  # noqa: F401
except Exception:  # pragma: no cover
    sys.path.insert(0, "/opt/trn_rl_repo")

import concourse.bass as bass
import concourse.mybir as mybir
from concourse.bass_utils import run_bass_kernel_spmd
from concourse.tile import TileContext
from concourse.vector_clock import ScopedClock

N_NODES = 50000
N_EDGES = 800000
H = 256
N_CORES = 8
NT = 49                 # node tiles per core
NPC = NT * 128          # 6272 nodes per core
N_PAD = N_CORES * NPC   # 50176
P = 128

FP8 = mybir.dt.float8e4
FP8_NP = mybir.dt.np(FP8)
BF16 = mybir.dt.bfloat16
BF16_NP = mybir.dt.np(BF16)
F32 = mybir.dt.float32
AF = mybir.ActivationFunctionType
OP = mybir.AluOpType
DR = mybir.MatmulPerfMode.DoubleRow

MM1_N = 512             # edges per DoubleRow mm1 matmul (rhs free = 2N)
WSCALE = 16.0           # fp8 prescale on W1 and W2


# ---------------------------------------------------------------------------
# Walrus on this toolchain rejects >2 sync waits on the TileContext tail
# drain ("Too many sync wait commands"); split them across SP NOPs.
def _patched_drain_and_barrier(self, tick_clock, wait_clock):
    drain_inst = self.nc.sync.drain()
    wait_clock.add_sem_waits(
        drain_inst.ins, ScopedClock({None: tick_clock.global_clock})
    )
    si = drain_inst.ins.sync_info
    if si is not None and si.on_wait and len(si.on_wait) > 1:
        extra = list(si.on_wait[1:])
        del si.on_wait[1:]
        for w in extra:
            nop = self.nc.sync.nop(nofuse=True, hint="drain_wait_split")
            nsi = nop.ins.sync_info
            if nsi is None:
                nop.ins.sync_info = mybir.SyncInfo(on_wait=[w], on_update=[])
            else:
                nsi.on_wait.append(w)

    self.nc.all_engine_barrier()
    assert self.sems is not None
    popped = self.nc._tile_sem_poison_stack.pop()
    assert popped is self._sem_poison
    self.nc.clear_and_free_semaphores(list(self.sems.allocated().values()))
    self.nc.all_engine_barrier()


TileContext._drain_and_barrier = _patched_drain_and_barrier


def _split_excess_waits(nc, maxw: int = 1):
    """Walrus rejects >maxw sync waits on one instruction; move the excess
    onto NOPs inserted just before, on the same engine (same-queue program
    order makes this equivalent)."""
    n_split = 0
    for f in nc.m.functions:
        for b in f.blocks:
            out = []
            for inst in b.instructions:
                si = inst.sync_info
                if si is not None and si.on_wait and len(si.on_wait) > maxw:
                    extra = list(si.on_wait[: -maxw])
                    del si.on_wait[: -maxw]
                    for i in range(0, len(extra), maxw):
                        nop = mybir.InstNoOp(
                            name=f"{inst.name}-wsplit{i}",
                            engine=inst.engine,
                            sync_info=mybir.SyncInfo(
                                on_wait=extra[i:i + maxw], on_update=[]),
                            bass_nofuse=True,
                        )
                        out.append(nop)
                    n_split += 1
                out.append(inst)
            b.instructions[:] = out
    return n_split
# ---------------------------------------------------------------------------


def _build_program(K: int):
    """Trace the single-core SPMD program for a fixed K (edge tiles per
    node-tile group)."""
    ET = NT * K                      # edge tiles per core
    n_mac = (ET + 7) // 8            # macros of up to 8 edge tiles
    n_sup = (n_mac + 1) // 2         # supertiles of 2 macros (1 DMA each)

    nc = bass.Bass()

    mijT = nc.dram_tensor("mijT", [n_sup, P, 4096], FP8, kind="ExternalInput")
    relw_d = nc.dram_tensor("relw", [P, ET * 4], F32, kind="ExternalInput")
    hT = nc.dram_tensor("hT", [NT, P, 2 * P], BF16, kind="ExternalInput")
    velg_d = nc.dram_tensor("velg", [P, NT * 18], F32, kind="ExternalInput")
    w1dr_d = nc.dram_tensor("w1dr", [2, P, 2 * P], FP8, kind="ExternalInput")
    w2t_d = nc.dram_tensor("w2t", [2, P, 1], FP8, kind="ExternalInput")
    b1t_d = nc.dram_tensor("b1t", [2, P, 1], F32, kind="ExternalInput")
    vgw1b_d = nc.dram_tensor("vgw1b", [2, 2, P, P], BF16, kind="ExternalInput")
    vgw2t_d = nc.dram_tensor("vgw2t", [2, P, 5], BF16, kind="ExternalInput")
    vgb1t_d = nc.dram_tensor("vgb1t", [2, P, 1], F32, kind="ExternalInput")
    out_d = nc.dram_tensor("out", [P, NT * 3], F32, kind="ExternalOutput")

    with TileContext(nc) as tc:
        with (
            tc.tile_pool(name="const", bufs=1) as cpool,
            tc.tile_pool(name="rhs", bufs=3) as rhs_pool,
            tc.tile_pool(name="s1", bufs=3) as s1_pool,
            tc.tile_pool(name="small", bufs=6) as sm_pool,
            tc.tile_pool(name="oh", bufs=4) as oh_pool,
            tc.tile_pool(name="nodes", bufs=3) as nd_pool,
            tc.tile_pool(name="ps_mm1", bufs=2, space="PSUM") as ps1_pool,
            tc.tile_pool(name="ps_w", bufs=2, space="PSUM") as psw_pool,
            tc.tile_pool(name="ps_sc", bufs=2, space="PSUM") as pssc_pool,
        ):
            # ---- constants ----
            w1 = [cpool.tile([P, 2 * P], FP8, tag=f"w1_{hh}", name=f"w1_{hh}")
                  for hh in range(2)]
            for hh in range(2):
                nc.sync.dma_start(w1[hh][:], w1dr_d[hh, :, :])
            vgw1 = [[cpool.tile([P, P], BF16, tag=f"vgw1_{kk}{hh}",
                                name=f"vgw1_{kk}{hh}")
                     for hh in range(2)] for kk in range(2)]
            for kk in range(2):
                for hh in range(2):
                    nc.sync.dma_start(vgw1[kk][hh][:], vgw1b_d[kk, hh, :, :])
            w2 = [cpool.tile([P, 1], FP8, tag=f"w2_{hh}", name=f"w2_{hh}")
                  for hh in range(2)]
            b1 = [cpool.tile([P, 1], F32, tag=f"b1_{hh}", name=f"b1_{hh}")
                  for hh in range(2)]
            vgw2 = [cpool.tile([P, 5], BF16, tag=f"vgw2_{hh}", name=f"vgw2_{hh}")
                    for hh in range(2)]
            vgb1 = [cpool.tile([P, 1], F32, tag=f"vgb1_{hh}", name=f"vgb1_{hh}")
                    for hh in range(2)]
            for hh in range(2):
                nc.sync.dma_start(w2[hh][:], w2t_d[hh, :, :])
                nc.sync.dma_start(b1[hh][:], b1t_d[hh, :, :])
                nc.sync.dma_start(vgw2[hh][:], vgw2t_d[hh, :, :])
                nc.sync.dma_start(vgb1[hh][:], vgb1t_d[hh, :, :])

            # iota values 0..127 repeated 8x: one-hot build for a whole macro
            iota = cpool.tile([P, 8 * P], BF16, tag="iota")
            nc.gpsimd.iota(iota[:], pattern=[[0, 8], [1, P]], base=0,
                           channel_multiplier=0,
                           allow_small_or_imprecise_dtypes=True)

            relw = cpool.tile([P, ET * 4], F32, tag="relw")
            nc.scalar.dma_start(relw[:], relw_d[:, :])
            velg = cpool.tile([P, NT * 18], F32, tag="velg")
            nc.scalar.dma_start(velg[:], velg_d[:, :])

            # packed output, one column triple per node tile; single DMA at end
            outbuf = cpool.tile([P, NT * 3], F32, tag="outbuf")

            # ---- PE warmup: ~4us of dummy matmuls off the iota tile so the
            # HAM clock gate reaches 8/8 before the real work lands ----
            ps_warm = ps1_pool.tile([P, 512], F32, tag="ps_mm1", name="warm")
            for _ in range(18):
                nc.tensor.matmul(ps_warm[:], iota[:, 0:P], iota[:, 0:512],
                                 start=True, stop=True)

            # node-tail emission, called when node tile nt's scatter psum is
            # fully accumulated
            def node_tail(nt: int, ps_sc):
                rhn = nd_pool.tile([P, 2 * P], BF16, tag="rhn")
                eng = nc.sync if nt % 2 == 0 else nc.scalar
                eng.dma_start(rhn[:], hT[nt, :, :])
                psn = [ps1_pool.tile([P, P], F32, tag="ps_mm1", name="psn")
                       for _ in range(2)]
                for hh in range(2):
                    for kk in range(2):
                        nc.tensor.matmul(psn[hh][:], vgw1[kk][hh][:],
                                         rhn[:, kk * P:(kk + 1) * P],
                                         start=(kk == 0), stop=(kk == 1))
                s1n = [nd_pool.tile([P, P], BF16, tag=f"s1n_{hh}",
                                    name=f"s1n_{hh}")
                       for hh in range(2)]
                for hh in range(2):
                    nc.scalar.activation(s1n[hh][:], psn[hh][:], AF.Silu,
                                         bias=vgb1[hh][:, 0:1], scale=1.0)
                psa = psw_pool.tile([P, 8], F32, tag="ps_w")
                for hh in range(2):
                    nc.tensor.matmul(psa[:, 0:5], s1n[hh][:], vgw2[hh][:],
                                     start=(hh == 0), stop=(hh == 1))

                # out[:, j] = ps_sc[:, j] + hostc[:, j] + sum_k alpha[k]*vel[j,k]
                scratch = sm_pool.tile([P, 15], F32, tag="scratch")
                vbase = nt * 18
                velg_v = velg[:, vbase:vbase + 15].rearrange(
                    "p (j k) -> p j k", k=5)
                nc.vector.tensor_tensor(
                    scratch[:].rearrange("p (j k) -> p j k", k=5),
                    velg_v,
                    psa[:, None, 0:5].broadcast_to([P, 3, 5]),
                    op=OP.mult)
                acc = sm_pool.tile([P, 3], F32, tag="acc")
                nc.vector.tensor_reduce(
                    acc[:, :, None],
                    scratch[:].rearrange("p (j k) -> p j k", k=5),
                    axis=mybir.AxisListType.X, op=OP.add)
                geomh = sm_pool.tile([P, 3], F32, tag="geomh")
                nc.vector.tensor_tensor(geomh[:], ps_sc[:, 0:3],
                                        velg[:, vbase + 15:vbase + 18],
                                        op=OP.add)
                nc.vector.tensor_add(outbuf[:, nt * 3:nt * 3 + 3],
                                     acc[:], geomh[:])

            # ---- edge-path macro loop (8 edge tiles per macro) ----
            ps_sc = None
            sup = None
            for m in range(n_mac):
                t0 = m * 8
                G = min(8, ET - t0)          # real edge tiles in this macro
                W = G * P                    # macro width in edges
                si, sm = divmod(m, 2)
                if sm == 0:
                    sup = rhs_pool.tile([P, 4096], FP8, tag="sup", name="sup")
                    eng = nc.sync if si % 2 == 0 else nc.scalar
                    eng.dma_start(sup[:], mijT[si, :, :])
                # [P, 2, 2048]: dim1 = k-half, dim2 = edge within supertile
                sup_v = sup[:].rearrange("p (i e) -> p i e", i=2)
                ps1 = [ps1_pool.tile([P, 1024], F32, tag="ps_mm1", name="ps1")
                       for _ in range(2)]
                n_ch = (W + MM1_N - 1) // MM1_N
                for hh in range(2):
                    for ch in range(n_ch):
                        cw = min(MM1_N, W - ch * MM1_N)
                        lo = sm * 1024 + ch * MM1_N
                        nc.tensor.matmul(
                            ps1[hh][:, ch * MM1_N:ch * MM1_N + cw],
                            w1[hh][:].rearrange("p (i m) -> p i m", i=2),
                            sup_v[:, :, lo:lo + cw],
                            start=True, stop=True, perf_mode=DR)
                s1 = [s1_pool.tile([P, 1024], FP8, tag=f"s1_{hh}",
                                   name=f"s1_{hh}")
                      for hh in range(2)]
                for hh in range(2):
                    nc.scalar.activation(s1[hh][:, 0:W], ps1[hh][:, 0:W],
                                         AF.Silu,
                                         bias=b1[hh][:, 0:1],
                                         scale=1.0 / WSCALE)
                psw = psw_pool.tile([P, 8], F32, tag="ps_w")
                for c in range(G):
                    for hh in range(2):
                        nc.tensor.matmul(psw[:, c:c + 1],
                                         s1[hh][:, c * P:(c + 1) * P],
                                         w2[hh][:],
                                         start=(hh == 0), stop=(hh == 1))

                relw_v = relw.rearrange("p (t f) -> p t f", f=4)
                # msg for all chunks in one op: [P, G, 3] = psw * rel'
                msg = sm_pool.tile([P, 24], BF16, tag="msg")
                nc.vector.tensor_tensor(
                    msg[:, 0:3 * G].rearrange("p (c f) -> p c f", f=3),
                    psw[:, 0:G, None].broadcast_to([P, G, 3]),
                    relw_v[:, t0:t0 + G, 0:3],
                    op=OP.mult)
                # one-hot for all chunks in one op: [P, G*128]
                oh = oh_pool.tile([P, 8 * P], FP8, tag="oh")
                nc.vector.tensor_tensor(
                    oh[:, 0:W].rearrange("p (c f) -> p c f", f=P),
                    iota[:, 0:W].rearrange("p (c f) -> p c f", f=P),
                    relw_v[:, t0:t0 + G, 3:4].broadcast_to([P, G, P]),
                    op=OP.is_equal)

                for c in range(G):
                    t = t0 + c
                    nt, j = divmod(t, K)
                    if j == 0:
                        ps_sc = pssc_pool.tile([P, 3], F32, tag="ps_sc")
                    nc.tensor.matmul(ps_sc[:], oh[:, c * P:(c + 1) * P],
                                     msg[:, 3 * c:3 * c + 3],
                                     start=(j == 0), stop=(j == K - 1))
                    if j == K - 1:
                        node_tail(nt, ps_sc)

            nc.sync.dma_start(out_d[:, :], outbuf[:])

    _split_excess_waits(nc)
    return nc


def _preprocess(inputs: dict):
    """Shard + lay out all per-core device inputs. Returns (in_maps, K)."""
    h = np.asarray(inputs["h"], np.float32)
    m_ij = np.asarray(inputs["m_ij"], np.float32)
    x = np.asarray(inputs["x"], np.float32)
    vel_all = np.asarray(inputs["vel_all"], np.float32)
    ei = np.asarray(inputs["edge_index"])
    src = ei[0].astype(np.int64)
    dst = ei[1].astype(np.int64)

    counts = np.bincount(dst, minlength=N_NODES).astype(np.float32)
    invc = (1.0 / np.maximum(counts, 1.0)).astype(np.float32)

    order = np.argsort(dst, kind="stable")
    dst_s = dst[order]
    src_s = src[order]
    g = dst_s // P                       # global 128-node group, 0..391
    n_groups = N_PAD // P                # 392
    cg = np.bincount(g, minlength=n_groups)
    K = max(1, int(-(-cg.max() // P)))   # ceil(max group)/128
    ET = NT * K

    gstart = np.zeros(n_groups, np.int64)
    gstart[1:] = np.cumsum(cg)[:-1]
    within = np.arange(N_EDGES, dtype=np.int64) - gstart[g]
    slot = g * (K * P) + within          # slot in global [392, K*128] layout

    rel = x[src_s] - x[dst_s]            # [E, 3] in dst-sorted order

    Sg = n_groups * K * P
    colidx = np.full(Sg, -1.0, np.float32)
    colidx[slot] = (dst_s % P).astype(np.float32)
    # rel' folds the scatter mean (invc) and the x16 prescale of W2
    relp = np.zeros((Sg, 3), np.float32)
    relp[slot] = rel * (invc[dst_s] / WSCALE)[:, None]
    mijp = np.zeros((Sg, H), FP8_NP)
    mijp[slot] = m_ij[order].astype(FP8_NP)

    # weights (shared by all cores)
    w1 = np.asarray(inputs["ew_W1"], np.float32)
    b1 = np.asarray(inputs["ew_b1"], np.float32)
    w2 = np.asarray(inputs["ew_W2"], np.float32)
    b2 = float(np.asarray(inputs["ew_b2"], np.float32)[0])
    vgw1 = np.asarray(inputs["vg_W1"], np.float32)
    vgb1 = np.asarray(inputs["vg_b1"], np.float32)
    vgw2 = np.asarray(inputs["vg_W2"], np.float32)
    vgb2 = np.asarray(inputs["vg_b2"], np.float32)

    # host-side constant term per node:
    #   b2 * invc * segsum(rel)  +  sum_k vgb2_k * vel_all[:, k, :]
    g2 = np.stack([np.bincount(dst, weights=rel_c, minlength=N_NODES)
                   for rel_c in (x[src, 0] - x[dst, 0],
                                 x[src, 1] - x[dst, 1],
                                 x[src, 2] - x[dst, 2])], axis=1)
    hostc = b2 * invc[:, None] * g2.astype(np.float32)
    hostc += np.einsum("k,nkj->nj", vgb2, vel_all).astype(np.float32)

    # w1dr[hh][p, i, m] = 16*W1[i*128+p, hh*128+m]
    w1dr = np.ascontiguousarray(
        (WSCALE * w1).reshape(2, P, 2, P).transpose(2, 1, 0, 3)
    ).astype(FP8_NP).reshape(2, P, 2 * P)
    w2t = (WSCALE * w2).reshape(2, P, 1).astype(FP8_NP).copy()
    b1t = b1.reshape(2, P, 1).copy()
    vgw1b = vgw1.reshape(2, P, 2, P).transpose(0, 2, 1, 3).astype(
        BF16_NP).copy()
    vgw2t = vgw2.reshape(2, P, 5).astype(BF16_NP).copy()
    vgb1t = vgb1.reshape(2, P, 1).copy()

    # padded node tensors
    hp = np.zeros((N_PAD, H), np.float32)
    hp[:N_NODES] = h
    velp = np.zeros((N_PAD, 5, 3), np.float32)
    velp[:N_NODES] = vel_all
    hostp = np.zeros((N_PAD, 3), np.float32)
    hostp[:N_NODES] = hostc

    mijp = mijp.reshape(N_CORES, ET * P, H)
    relp = relp.reshape(N_CORES, ET, P, 3)
    colidx = colidx.reshape(N_CORES, ET, P)

    n_mac = (ET + 7) // 8
    n_sup = (n_mac + 1) // 2
    in_maps = []
    for k in range(N_CORES):
        # mijT supertiles: [n_sup, 128, 4096] fp8 DoubleRow layout:
        # [p, i*2048 + e] = m[sup_base + e, i*128 + p]
        full = np.zeros((n_sup * 2048, 2, P), FP8_NP)
        full[:ET * P] = mijp[k].reshape(ET * P, 2, P)
        mijT = np.ascontiguousarray(
            full.reshape(n_sup, 2048, 2, P).transpose(0, 3, 2, 1)
        ).reshape(n_sup, P, 4096)

        rw = np.empty((P, ET, 4), np.float32)
        rw[:, :, 0:3] = relp[k].transpose(1, 0, 2)
        rw[:, :, 3] = colidx[k].T
        relw = np.ascontiguousarray(rw).reshape(P, ET * 4)

        hk = hp[k * NPC:(k + 1) * NPC].reshape(NT, P, H)
        hTk = np.ascontiguousarray(
            hk.transpose(0, 2, 1).reshape(NT, 2, P, P).transpose(0, 2, 1, 3)
        ).reshape(NT, P, 2 * P).astype(BF16_NP)

        # velg cols per node tile: [comp j, gate k] at 5*j+k, hostc at 15:18
        vg = np.empty((P, NT, 18), np.float32)
        vg[:, :, 0:15] = (velp[k * NPC:(k + 1) * NPC]
                          .reshape(NT, P, 5, 3).transpose(1, 0, 3, 2)
                          .reshape(P, NT, 15))
        vg[:, :, 15:18] = (hostp[k * NPC:(k + 1) * NPC]
                           .reshape(NT, P, 3).transpose(1, 0, 2))
        velg = np.ascontiguousarray(vg).reshape(P, NT * 18)

        in_maps.append({
            "mijT": mijT,
            "relw": relw,
            "hT": hTk,
            "velg": velg,
            "w1dr": w1dr,
            "w2t": w2t,
            "b1t": b1t,
            "vgw1b": vgw1b,
            "vgw2t": vgw2t,
            "vgb1t": vgb1t,
        })
    return in_maps, K


def unpack_out(arr: np.ndarray) -> np.ndarray:
    """[128, NT*3] packed per-core output -> [NPC, 3]."""
    return arr.reshape(P, NT, 3).transpose(1, 0, 2).reshape(NPC, 3)


def kernel(**inputs) -> np.ndarray:
    in_maps, K = _preprocess(inputs)
    nc = _build_program(K)
    res = run_bass_kernel_spmd(nc, in_maps, list(range(N_CORES)))
    parts = [unpack_out(res.results[k]["out"]) for k in range(N_CORES)]
    return np.concatenate(parts, axis=0)[:N_NODES].astype(np.float32)


# revision 5
# speedup vs baseline: 1.1703x; 1.0476x over previous
"""Trainium2 Bass kernel for nn_EquivariantDecoder (GNN message passing).

Sharding: nodes are split into 8 contiguous ranges of 6272 (= 49 tiles of
128); each core owns the edges whose dst lands in its range, so per-node
segment sums are core-local (no collectives). Edges are sorted by dst on
the host and padded so every (core, node-tile) group holds exactly K
tiles of 128 edge slots; the K is baked into the traced program.

v3 layout (fp8 edge path):
  mm1:  z = W1.T @ m_ij, fp8 non-DoubleRow (DR serializes its 256-col
        LDWEIGHTS; non-DR FWL loads hide under the 512-col streams),
        weights pre-scaled x16, un-scaled via the silu's input scale
  silu: ACT, out in fp8 (s1), scale=1/16 folds the weight prescale
  mm2:  w = s1.T @ W2 per edge tile, fp8 stationary (fast weight load)
  scatter: one-hot matmul per edge tile (oh in fp8, msg in bf16);
        1/count, x16 of W2, and the b2*rel term are all folded on host:
        msg = psw * rel' with rel' = rel * invc[dst] / 16
  node: bf16 MLP hoisted off the critical path - pairs of node tiles are
        processed alongside the first 25 edge macros with prefetched h
        DMAs; vel_combo + host constant (velb2 + b2-geom) land in outbuf
        and the scatter tail is a single DVE add.
"""

import sys

import numpy as np

try:
    import concourse.bass as bass  # noqa: F401
except Exception:  # pragma: no cover
    sys.path.insert(0, "/opt/trn_rl_repo")

import concourse.bass as bass
import concourse.mybir as mybir
from concourse.bass_utils import run_bass_kernel_spmd
from concourse.tile import TileContext
from concourse.vector_clock import ScopedClock

N_NODES = 50000
N_EDGES = 800000
H = 256
N_CORES = 8
NT = 49                 # node tiles per core
NPC = NT * 128          # 6272 nodes per core
N_PAD = N_CORES * NPC   # 50176
P = 128

FP8 = mybir.dt.float8e4
FP8_NP = mybir.dt.np(FP8)
BF16 = mybir.dt.bfloat16
BF16_NP = mybir.dt.np(BF16)
F32 = mybir.dt.float32
AF = mybir.ActivationFunctionType
OP = mybir.AluOpType
DR = mybir.MatmulPerfMode.DoubleRow

MM1_N = 512             # edges per DoubleRow mm1 matmul (rhs free = 2N)
WSCALE = 16.0           # fp8 prescale on W1 and W2


# ---------------------------------------------------------------------------
# Walrus on this toolchain rejects >2 sync waits on the TileContext tail
# drain ("Too many sync wait commands"); split them across SP NOPs.
def _patched_drain_and_barrier(self, tick_clock, wait_clock):
    drain_inst = self.nc.sync.drain()
    wait_clock.add_sem_waits(
        drain_inst.ins, ScopedClock({None: tick_clock.global_clock})
    )
    si = drain_inst.ins.sync_info
    if si is not None and si.on_wait and len(si.on_wait) > 1:
        extra = list(si.on_wait[1:])
        del si.on_wait[1:]
        for w in extra:
            nop = self.nc.sync.nop(nofuse=True, hint="drain_wait_split")
            nsi = nop.ins.sync_info
            if nsi is None:
                nop.ins.sync_info = mybir.SyncInfo(on_wait=[w], on_update=[])
            else:
                nsi.on_wait.append(w)

    self.nc.all_engine_barrier()
    assert self.sems is not None
    popped = self.nc._tile_sem_poison_stack.pop()
    assert popped is self._sem_poison
    self.nc.clear_and_free_semaphores(list(self.sems.allocated().values()))
    self.nc.all_engine_barrier()


TileContext._drain_and_barrier = _patched_drain_and_barrier


def _split_excess_waits(nc, maxw: int = 1):
    """Walrus rejects >maxw sync waits on one instruction; move the excess
    onto NOPs inserted just before, on the same engine (same-queue program
    order makes this equivalent)."""
    n_split = 0
    for f in nc.m.functions:
        for b in f.blocks:
            out = []
            for inst in b.instructions:
                si = inst.sync_info
                if si is not None and si.on_wait and len(si.on_wait) > maxw:
                    extra = list(si.on_wait[: -maxw])
                    del si.on_wait[: -maxw]
                    for i in range(0, len(extra), maxw):
                        nop = mybir.InstNoOp(
                            name=f"{inst.name}-wsplit{i}",
                            engine=inst.engine,
                            sync_info=mybir.SyncInfo(
                                on_wait=extra[i:i + maxw], on_update=[]),
                            bass_nofuse=True,
                        )
                        out.append(nop)
                    n_split += 1
                out.append(inst)
            b.instructions[:] = out
    return n_split
# ---------------------------------------------------------------------------


def _build_program(K: int):
    """Trace the single-core SPMD program for a fixed K (edge tiles per
    node-tile group)."""
    ET = NT * K                      # edge tiles per core
    n_mac = (ET + 7) // 8            # macros of up to 8 edge tiles
    n_sup = (n_mac + 1) // 2         # supertiles of 2 macros (1 DMA each)

    nc = bass.Bass()

    mijT = nc.dram_tensor("mijT", [n_sup, P, 4096], FP8, kind="ExternalInput")
    relw_d = nc.dram_tensor("relw", [P, ET * 4], F32, kind="ExternalInput")
    hT = nc.dram_tensor("hT", [NT, P, 2 * P], BF16, kind="ExternalInput")
    velg_d = nc.dram_tensor("velg", [P, NT * 18], F32, kind="ExternalInput")
    w1dr_d = nc.dram_tensor("w1dr", [2, P, 2 * P], FP8, kind="ExternalInput")
    w2t_d = nc.dram_tensor("w2t", [2, P, 1], FP8, kind="ExternalInput")
    b1t_d = nc.dram_tensor("b1t", [2, P, 1], F32, kind="ExternalInput")
    vgw1b_d = nc.dram_tensor("vgw1b", [2, 2, P, P], BF16, kind="ExternalInput")
    vgw2t_d = nc.dram_tensor("vgw2t", [2, P, 5], BF16, kind="ExternalInput")
    vgb1t_d = nc.dram_tensor("vgb1t", [2, P, 1], F32, kind="ExternalInput")
    out_d = nc.dram_tensor("out", [P, NT * 3], F32, kind="ExternalOutput")

    with TileContext(nc) as tc:
        with (
            tc.tile_pool(name="const", bufs=1) as cpool,
            tc.tile_pool(name="rhs", bufs=3) as rhs_pool,
            tc.tile_pool(name="s1", bufs=3) as s1_pool,
            tc.tile_pool(name="small", bufs=6) as sm_pool,
            tc.tile_pool(name="oh", bufs=4) as oh_pool,
            tc.tile_pool(name="nodes", bufs=3) as nd_pool,
            tc.tile_pool(name="ps_mm1", bufs=2, space="PSUM") as ps1_pool,
            tc.tile_pool(name="ps_w", bufs=2, space="PSUM") as psw_pool,
            tc.tile_pool(name="ps_sc", bufs=2, space="PSUM") as pssc_pool,
        ):
            # ---- constants ----
            w1 = [cpool.tile([P, 2 * P], FP8, tag=f"w1_{hh}", name=f"w1_{hh}")
                  for hh in range(2)]
            for hh in range(2):
                nc.sync.dma_start(w1[hh][:], w1dr_d[hh, :, :])
            vgw1 = [[cpool.tile([P, P], BF16, tag=f"vgw1_{kk}{hh}",
                                name=f"vgw1_{kk}{hh}")
                     for hh in range(2)] for kk in range(2)]
            for kk in range(2):
                for hh in range(2):
                    nc.sync.dma_start(vgw1[kk][hh][:], vgw1b_d[kk, hh, :, :])
            w2 = [cpool.tile([P, 1], FP8, tag=f"w2_{hh}", name=f"w2_{hh}")
                  for hh in range(2)]
            b1 = [cpool.tile([P, 1], F32, tag=f"b1_{hh}", name=f"b1_{hh}")
                  for hh in range(2)]
            vgw2 = [cpool.tile([P, 5], BF16, tag=f"vgw2_{hh}", name=f"vgw2_{hh}")
                    for hh in range(2)]
            vgb1 = [cpool.tile([P, 1], F32, tag=f"vgb1_{hh}", name=f"vgb1_{hh}")
                    for hh in range(2)]
            for hh in range(2):
                nc.sync.dma_start(w2[hh][:], w2t_d[hh, :, :])
                nc.sync.dma_start(b1[hh][:], b1t_d[hh, :, :])
                nc.sync.dma_start(vgw2[hh][:], vgw2t_d[hh, :, :])
                nc.sync.dma_start(vgb1[hh][:], vgb1t_d[hh, :, :])

            # iota values 0..127 repeated 8x: one-hot build for a whole macro
            iota = cpool.tile([P, 8 * P], BF16, tag="iota")
            nc.gpsimd.iota(iota[:], pattern=[[0, 8], [1, P]], base=0,
                           channel_multiplier=0,
                           allow_small_or_imprecise_dtypes=True)

            relw = cpool.tile([P, ET * 4], F32, tag="relw")
            nc.scalar.dma_start(relw[:], relw_d[:, :])
            velg = cpool.tile([P, NT * 18], F32, tag="velg")
            nc.scalar.dma_start(velg[:], velg_d[:, :])

            # packed output, one column triple per node tile; single DMA at end
            outbuf = cpool.tile([P, NT * 3], F32, tag="outbuf")

            # ---- PE warmup: ~4us of dummy matmuls off the iota tile so the
            # HAM clock gate reaches 8/8 before the real work lands ----
            ps_warm = ps1_pool.tile([P, 512], F32, tag="ps_mm1", name="warm")
            for _ in range(18):
                nc.tensor.matmul(ps_warm[:], iota[:, 0:P], iota[:, 0:512],
                                 start=True, stop=True)

            # ---- node-path MLP, hoisted: pairs of node tiles, alpha ->
            # vel_combo + hostc lands in outbuf ahead of the scatter tail ----
            rhn_tiles = {}

            def node_fetch(pr: int):
                lo = 2 * pr
                nt_n = min(2, NT - lo)
                rhn = nd_pool.tile([P, nt_n * 2 * P], BF16, tag="rhn")
                eng = nc.sync if pr % 2 == 0 else nc.scalar
                eng.dma_start(
                    rhn[:].rearrange("p (t c) -> p t c", t=nt_n),
                    hT[lo:lo + nt_n, :, :].rearrange("t p c -> p t c"))
                rhn_tiles[pr] = rhn

            def node_pre(pr: int):
                lo = 2 * pr
                nt_n = min(2, NT - lo)
                rhn = rhn_tiles.pop(pr)
                rhn_v = rhn[:].rearrange("p (t k c) -> p t k c", t=nt_n, k=2)
                psn = [ps1_pool.tile([P, nt_n * P], F32, tag="ps_mm1",
                                     name="psn")
                       for _ in range(2)]
                for hh in range(2):
                    for kk in range(2):
                        nc.tensor.matmul(psn[hh][:], vgw1[kk][hh][:],
                                         rhn_v[:, :, kk, :],
                                         start=(kk == 0), stop=(kk == 1))
                s1n = [nd_pool.tile([P, nt_n * P], BF16, tag=f"s1n_{hh}",
                                    name=f"s1n_{hh}")
                       for hh in range(2)]
                for hh in range(2):
                    nc.scalar.activation(s1n[hh][:], psn[hh][:], AF.Silu,
                                         bias=vgb1[hh][:, 0:1], scale=1.0)
                for t in range(nt_n):
                    nt = lo + t
                    psa = psw_pool.tile([P, 8], F32, tag="ps_w")
                    for hh in range(2):
                        nc.tensor.matmul(psa[:, 0:5],
                                         s1n[hh][:, t * P:(t + 1) * P],
                                         vgw2[hh][:],
                                         start=(hh == 0), stop=(hh == 1))
                    # outbuf[:, j] = hostc[:, j] + sum_k alpha[k] * vel[j, k]
                    scratch = sm_pool.tile([P, 15], F32, tag="scratch")
                    vbase = nt * 18
                    velg_v = velg[:, vbase:vbase + 15].rearrange(
                        "p (j k) -> p j k", k=5)
                    nc.vector.tensor_tensor(
                        scratch[:].rearrange("p (j k) -> p j k", k=5),
                        velg_v,
                        psa[:, None, 0:5].broadcast_to([P, 3, 5]),
                        op=OP.mult)
                    acc = sm_pool.tile([P, 3], F32, tag="acc")
                    nc.vector.tensor_reduce(
                        acc[:, :, None],
                        scratch[:].rearrange("p (j k) -> p j k", k=5),
                        axis=mybir.AxisListType.X, op=OP.add)
                    nc.vector.tensor_add(outbuf[:, nt * 3:nt * 3 + 3],
                                         acc[:],
                                         velg[:, vbase + 15:vbase + 18])

            def node_tail(nt: int, ps_sc):
                nc.vector.tensor_add(outbuf[:, nt * 3:nt * 3 + 3],
                                     ps_sc[:, 0:3],
                                     outbuf[:, nt * 3:nt * 3 + 3])

            n_pair = (NT + 1) // 2
            node_fetch(0)
            node_fetch(1)

            # ---- edge-path macro loop (8 edge tiles per macro) ----
            ps_sc = None
            sup = None
            for m in range(n_mac):
                if m + 2 < n_pair:
                    node_fetch(m + 2)
                t0 = m * 8
                G = min(8, ET - t0)          # real edge tiles in this macro
                W = G * P                    # macro width in edges
                si, sm = divmod(m, 2)
                if sm == 0:
                    sup = rhs_pool.tile([P, 4096], FP8, tag="sup", name="sup")
                    eng = nc.sync if si % 2 == 0 else nc.scalar
                    eng.dma_start(sup[:], mijT[si, :, :])
                # [P, 2, 2048]: dim1 = k-half, dim2 = edge within supertile
                sup_v = sup[:].rearrange("p (i e) -> p i e", i=2)
                ps1 = [ps1_pool.tile([P, 1024], F32, tag="ps_mm1", name="ps1")
                       for _ in range(2)]
                n_ch = (W + MM1_N - 1) // MM1_N
                w1_v = [w1[hh][:].rearrange("p (i m) -> p i m", i=2)
                        for hh in range(2)]
                for hh in range(2):
                    for ch in range(n_ch):
                        cw = min(MM1_N, W - ch * MM1_N)
                        lo = sm * 1024 + ch * MM1_N
                        for kk in range(2):
                            nc.tensor.matmul(
                                ps1[hh][:, ch * MM1_N:ch * MM1_N + cw],
                                w1_v[hh][:, kk, :],
                                sup_v[:, kk, lo:lo + cw],
                                start=(kk == 0), stop=(kk == 1))
                if m < n_pair:
                    node_pre(m)
                s1 = [s1_pool.tile([P, 1024], FP8, tag=f"s1_{hh}",
                                   name=f"s1_{hh}")
                      for hh in range(2)]
                for hh in range(2):
                    nc.scalar.activation(s1[hh][:, 0:W], ps1[hh][:, 0:W],
                                         AF.Silu,
                                         bias=b1[hh][:, 0:1],
                                         scale=1.0 / WSCALE)
                psw = psw_pool.tile([P, 8], F32, tag="ps_w")
                for c in range(G):
                    for hh in range(2):
                        nc.tensor.matmul(psw[:, c:c + 1],
                                         s1[hh][:, c * P:(c + 1) * P],
                                         w2[hh][:],
                                         start=(hh == 0), stop=(hh == 1))

                relw_v = relw.rearrange("p (t f) -> p t f", f=4)
                # msg for all chunks in one op: [P, G, 3] = psw * rel'
                msg = sm_pool.tile([P, 24], BF16, tag="msg")
                nc.vector.tensor_tensor(
                    msg[:, 0:3 * G].rearrange("p (c f) -> p c f", f=3),
                    psw[:, 0:G, None].broadcast_to([P, G, 3]),
                    relw_v[:, t0:t0 + G, 0:3],
                    op=OP.mult)
                # one-hot for all chunks in one op: [P, G*128]
                oh = oh_pool.tile([P, 8 * P], FP8, tag="oh")
                nc.vector.tensor_tensor(
                    oh[:, 0:W].rearrange("p (c f) -> p c f", f=P),
                    iota[:, 0:W].rearrange("p (c f) -> p c f", f=P),
                    relw_v[:, t0:t0 + G, 3:4].broadcast_to([P, G, P]),
                    op=OP.is_equal)

                for c in range(G):
                    t = t0 + c
                    nt, j = divmod(t, K)
                    if j == 0:
                        ps_sc = pssc_pool.tile([P, 3], F32, tag="ps_sc")
                    nc.tensor.matmul(ps_sc[:], oh[:, c * P:(c + 1) * P],
                                     msg[:, 3 * c:3 * c + 3],
                                     start=(j == 0), stop=(j == K - 1))
                    if j == K - 1:
                        node_tail(nt, ps_sc)

            nc.sync.dma_start(out_d[:, :], outbuf[:])

    _split_excess_waits(nc)
    return nc


def _preprocess(inputs: dict):
    """Shard + lay out all per-core device inputs. Returns (in_maps, K)."""
    h = np.asarray(inputs["h"], np.float32)
    m_ij = np.asarray(inputs["m_ij"], np.float32)
    x = np.asarray(inputs["x"], np.float32)
    vel_all = np.asarray(inputs["vel_all"], np.float32)
    ei = np.asarray(inputs["edge_index"])
    src = ei[0].astype(np.int64)
    dst = ei[1].astype(np.int64)

    counts = np.bincount(dst, minlength=N_NODES).astype(np.float32)
    invc = (1.0 / np.maximum(counts, 1.0)).astype(np.float32)

    order = np.argsort(dst, kind="stable")
    dst_s = dst[order]
    src_s = src[order]
    g = dst_s // P                       # global 128-node group, 0..391
    n_groups = N_PAD // P                # 392
    cg = np.bincount(g, minlength=n_groups)
    K = max(1, int(-(-cg.max() // P)))   # ceil(max group)/128
    ET = NT * K

    gstart = np.zeros(n_groups, np.int64)
    gstart[1:] = np.cumsum(cg)[:-1]
    within = np.arange(N_EDGES, dtype=np.int64) - gstart[g]
    slot = g * (K * P) + within          # slot in global [392, K*128] layout

    rel = x[src_s] - x[dst_s]            # [E, 3] in dst-sorted order

    Sg = n_groups * K * P
    colidx = np.full(Sg, -1.0, np.float32)
    colidx[slot] = (dst_s % P).astype(np.float32)
    # rel' folds the scatter mean (invc) and the x16 prescale of W2
    relp = np.zeros((Sg, 3), np.float32)
    relp[slot] = rel * (invc[dst_s] / WSCALE)[:, None]
    mijp = np.zeros((Sg, H), FP8_NP)
    mijp[slot] = m_ij[order].astype(FP8_NP)

    # weights (shared by all cores)
    w1 = np.asarray(inputs["ew_W1"], np.float32)
    b1 = np.asarray(inputs["ew_b1"], np.float32)
    w2 = np.asarray(inputs["ew_W2"], np.float32)
    b2 = float(np.asarray(inputs["ew_b2"], np.float32)[0])
    vgw1 = np.asarray(inputs["vg_W1"], np.float32)
    vgb1 = np.asarray(inputs["vg_b1"], np.float32)
    vgw2 = np.asarray(inputs["vg_W2"], np.float32)
    vgb2 = np.asarray(inputs["vg_b2"], np.float32)

    # host-side constant term per node:
    #   b2 * invc * segsum(rel)  +  sum_k vgb2_k * vel_all[:, k, :]
    g2 = np.stack([np.bincount(dst, weights=rel_c, minlength=N_NODES)
                   for rel_c in (x[src, 0] - x[dst, 0],
                                 x[src, 1] - x[dst, 1],
                                 x[src, 2] - x[dst, 2])], axis=1)
    hostc = b2 * invc[:, None] * g2.astype(np.float32)
    hostc += np.einsum("k,nkj->nj", vgb2, vel_all).astype(np.float32)

    # w1dr[hh][p, i, m] = 16*W1[i*128+p, hh*128+m]
    w1dr = np.ascontiguousarray(
        (WSCALE * w1).reshape(2, P, 2, P).transpose(2, 1, 0, 3)
    ).astype(FP8_NP).reshape(2, P, 2 * P)
    w2t = (WSCALE * w2).reshape(2, P, 1).astype(FP8_NP).copy()
    b1t = b1.reshape(2, P, 1).copy()
    vgw1b = vgw1.reshape(2, P, 2, P).transpose(0, 2, 1, 3).astype(
        BF16_NP).copy()
    vgw2t = vgw2.reshape(2, P, 5).astype(BF16_NP).copy()
    vgb1t = vgb1.reshape(2, P, 1).copy()

    # padded node tensors
    hp = np.zeros((N_PAD, H), np.float32)
    hp[:N_NODES] = h
    velp = np.zeros((N_PAD, 5, 3), np.float32)
    velp[:N_NODES] = vel_all
    hostp = np.zeros((N_PAD, 3), np.float32)
    hostp[:N_NODES] = hostc

    mijp = mijp.reshape(N_CORES, ET * P, H)
    relp = relp.reshape(N_CORES, ET, P, 3)
    colidx = colidx.reshape(N_CORES, ET, P)

    n_mac = (ET + 7) // 8
    n_sup = (n_mac + 1) // 2
    in_maps = []
    for k in range(N_CORES):
        # mijT supertiles: [n_sup, 128, 4096] fp8 DoubleRow layout:
        # [p, i*2048 + e] = m[sup_base + e, i*128 + p]
        full = np.zeros((n_sup * 2048, 2, P), FP8_NP)
        full[:ET * P] = mijp[k].reshape(ET * P, 2, P)
        mijT = np.ascontiguousarray(
            full.reshape(n_sup, 2048, 2, P).transpose(0, 3, 2, 1)
        ).reshape(n_sup, P, 4096)

        rw = np.empty((P, ET, 4), np.float32)
        rw[:, :, 0:3] = relp[k].transpose(1, 0, 2)
        rw[:, :, 3] = colidx[k].T
        relw = np.ascontiguousarray(rw).reshape(P, ET * 4)

        hk = hp[k * NPC:(k + 1) * NPC].reshape(NT, P, H)
        hTk = np.ascontiguousarray(
            hk.transpose(0, 2, 1).reshape(NT, 2, P, P).transpose(0, 2, 1, 3)
        ).reshape(NT, P, 2 * P).astype(BF16_NP)

        # velg cols per node tile: [comp j, gate k] at 5*j+k, hostc at 15:18
        vg = np.empty((P, NT, 18), np.float32)
        vg[:, :, 0:15] = (velp[k * NPC:(k + 1) * NPC]
                          .reshape(NT, P, 5, 3).transpose(1, 0, 3, 2)
                          .reshape(P, NT, 15))
        vg[:, :, 15:18] = (hostp[k * NPC:(k + 1) * NPC]
                           .reshape(NT, P, 3).transpose(1, 0, 2))
        velg = np.ascontiguousarray(vg).reshape(P, NT * 18)

        in_maps.append({
            "mijT": mijT,
            "relw": relw,
            "hT": hTk,
            "velg": velg,
            "w1dr": w1dr,
            "w2t": w2t,
            "b1t": b1t,
            "vgw1b": vgw1b,
            "vgw2t": vgw2t,
            "vgb1t": vgb1t,
        })
    return in_maps, K


def unpack_out(arr: np.ndarray) -> np.ndarray:
    """[128, NT*3] packed per-core output -> [NPC, 3]."""
    return arr.reshape(P, NT, 3).transpose(1, 0, 2).reshape(NPC, 3)


def kernel(**inputs) -> np.ndarray:
    in_maps, K = _preprocess(inputs)
    nc = _build_program(K)
    res = run_bass_kernel_spmd(nc, in_maps, list(range(N_CORES)))
    parts = [unpack_out(res.results[k]["out"]) for k in range(N_CORES)]
    return np.concatenate(parts, axis=0)[:N_NODES].astype(np.float32)


# revision 11
# speedup vs baseline: 1.2540x; 1.0715x over previous
"""Trainium2 Bass kernel for nn_EquivariantDecoder (GNN message passing).

Sharding: nodes are split into 8 contiguous ranges of 6272 (= 49 tiles of
128); each core owns the edges whose dst lands in its range, so per-node
segment sums are core-local (no collectives). Edges are sorted by dst on
the host and padded so every (core, node-tile) group holds exactly K
tiles of 128 edge slots; the K is baked into the traced program.

v3 layout (fp8 edge path):
  mm1:  z = W1.T @ m_ij, fp8 non-DoubleRow (DR serializes its 256-col
        LDWEIGHTS; non-DR FWL loads hide under the 512-col streams),
        weights pre-scaled x16, un-scaled via the silu's input scale
  silu: ACT, out in fp8 (s1), scale=1/16 folds the weight prescale
  mm2:  w = s1.T @ W2 per edge tile, fp8 stationary (fast weight load)
  scatter: one-hot matmul per edge tile (oh in fp8, msg in bf16);
        1/count, x16 of W2, and the b2*rel term are all folded on host:
        msg = psw * rel' with rel' = rel * invc[dst] / 16
  node: bf16 MLP hoisted off the critical path - pairs of node tiles are
        processed alongside the first 25 edge macros with prefetched h
        DMAs; vel_combo + host constant (velb2 + b2-geom) land in outbuf
        and the scatter tail is a single DVE add.
"""

import sys

import numpy as np

try:
    import concourse.bass as bass  # noqa: F401
except Exception:  # pragma: no cover
    sys.path.insert(0, "/opt/trn_rl_repo")

import concourse.bass as bass
import concourse.mybir as mybir
from concourse.bass_utils import run_bass_kernel_spmd
from concourse.tile import TileContext
from concourse.vector_clock import ScopedClock

N_NODES = 50000
N_EDGES = 800000
H = 256
N_CORES = 8
NT = 49                 # node tiles per core
NPC = NT * 128          # 6272 nodes per core
N_PAD = N_CORES * NPC   # 50176
P = 128

FP8 = mybir.dt.float8e4
FP8_NP = mybir.dt.np(FP8)
BF16 = mybir.dt.bfloat16
BF16_NP = mybir.dt.np(BF16)
F32 = mybir.dt.float32
AF = mybir.ActivationFunctionType
OP = mybir.AluOpType
DR = mybir.MatmulPerfMode.DoubleRow

MM1_N = 512             # edges per DoubleRow mm1 matmul (rhs free = 2N)
WSCALE = 16.0           # fp8 prescale on W1 and W2


# ---------------------------------------------------------------------------
# Walrus on this toolchain rejects >2 sync waits on the TileContext tail
# drain ("Too many sync wait commands"); split them across SP NOPs.
def _patched_drain_and_barrier(self, tick_clock, wait_clock):
    drain_inst = self.nc.sync.drain()
    wait_clock.add_sem_waits(
        drain_inst.ins, ScopedClock({None: tick_clock.global_clock})
    )
    si = drain_inst.ins.sync_info
    if si is not None and si.on_wait and len(si.on_wait) > 1:
        extra = list(si.on_wait[1:])
        del si.on_wait[1:]
        for w in extra:
            nop = self.nc.sync.nop(nofuse=True, hint="drain_wait_split")
            nsi = nop.ins.sync_info
            if nsi is None:
                nop.ins.sync_info = mybir.SyncInfo(on_wait=[w], on_update=[])
            else:
                nsi.on_wait.append(w)

    self.nc.all_engine_barrier()
    assert self.sems is not None
    popped = self.nc._tile_sem_poison_stack.pop()
    assert popped is self._sem_poison
    self.nc.clear_and_free_semaphores(list(self.sems.allocated().values()))
    self.nc.all_engine_barrier()


TileContext._drain_and_barrier = _patched_drain_and_barrier


def _split_excess_waits(nc, maxw: int = 1):
    """Walrus rejects >maxw sync waits on one instruction; move the excess
    onto NOPs inserted just before, on the same engine (same-queue program
    order makes this equivalent)."""
    n_split = 0
    for f in nc.m.functions:
        for b in f.blocks:
            out = []
            for inst in b.instructions:
                si = inst.sync_info
                if si is not None and si.on_wait and len(si.on_wait) > maxw:
                    extra = list(si.on_wait[: -maxw])
                    del si.on_wait[: -maxw]
                    for i in range(0, len(extra), maxw):
                        nop = mybir.InstNoOp(
                            name=f"{inst.name}-wsplit{i}",
                            engine=inst.engine,
                            sync_info=mybir.SyncInfo(
                                on_wait=extra[i:i + maxw], on_update=[]),
                            bass_nofuse=True,
                        )
                        out.append(nop)
                    n_split += 1
                out.append(inst)
            b.instructions[:] = out
    return n_split
# ---------------------------------------------------------------------------


def _build_program(K: int):
    """Trace the single-core SPMD program for a fixed K (edge tiles per
    node-tile group)."""
    ET = NT * K                      # edge tiles per core
    n_mac = (ET + 7) // 8            # macros of up to 8 edge tiles
    n_sup = (n_mac + 1) // 2         # supertiles of 2 macros (1 DMA each)

    nc = bass.Bass()

    mijT = nc.dram_tensor("mijT", [n_sup, P, 4096], FP8, kind="ExternalInput")
    relw_d = nc.dram_tensor("relw", [P, ET * 4], F32, kind="ExternalInput")
    hT = nc.dram_tensor("hT", [NT, P, 2 * P], BF16, kind="ExternalInput")
    velg_d = nc.dram_tensor("velg", [P, NT * 18], F32, kind="ExternalInput")
    w1dr_d = nc.dram_tensor("w1dr", [2, P, 2 * P], FP8, kind="ExternalInput")
    w2t_d = nc.dram_tensor("w2t", [2, P, 1], FP8, kind="ExternalInput")
    b1t_d = nc.dram_tensor("b1t", [2, P, 1], F32, kind="ExternalInput")
    vgw1b_d = nc.dram_tensor("vgw1b", [2, 2, P, P], BF16, kind="ExternalInput")
    vgw2t_d = nc.dram_tensor("vgw2t", [2, P, 5], BF16, kind="ExternalInput")
    vgb1t_d = nc.dram_tensor("vgb1t", [2, P, 1], F32, kind="ExternalInput")
    out_d = nc.dram_tensor("out", [P, NT * 3], F32, kind="ExternalOutput")

    with TileContext(nc) as tc:
        with (
            tc.tile_pool(name="const", bufs=1) as cpool,
            tc.tile_pool(name="rhs", bufs=3) as rhs_pool,
            tc.tile_pool(name="s1", bufs=3) as s1_pool,
            tc.tile_pool(name="small", bufs=6) as sm_pool,
            tc.tile_pool(name="oh", bufs=4) as oh_pool,
            tc.tile_pool(name="nodes", bufs=3) as nd_pool,
            tc.tile_pool(name="ps_mm1", bufs=2, space="PSUM") as ps1_pool,
            tc.tile_pool(name="ps_w", bufs=2, space="PSUM") as psw_pool,
            tc.tile_pool(name="ps_sc", bufs=2, space="PSUM") as pssc_pool,
        ):
            # ---- constants (small weight DMAs on the gpsimd/vector queues
            # so the sync/scalar queues start streaming mijT immediately) ----
            w1 = [cpool.tile([P, 2 * P], FP8, tag=f"w1_{hh}", name=f"w1_{hh}")
                  for hh in range(2)]
            for hh in range(2):
                nc.gpsimd.dma_start(w1[hh][:], w1dr_d[hh, :, :])
            vgw1 = [[cpool.tile([P, P], BF16, tag=f"vgw1_{kk}{hh}",
                                name=f"vgw1_{kk}{hh}")
                     for hh in range(2)] for kk in range(2)]
            for kk in range(2):
                for hh in range(2):
                    nc.gpsimd.dma_start(vgw1[kk][hh][:], vgw1b_d[kk, hh, :, :])
            w2 = [cpool.tile([P, 1], FP8, tag=f"w2_{hh}", name=f"w2_{hh}")
                  for hh in range(2)]
            b1 = [cpool.tile([P, 1], F32, tag=f"b1_{hh}", name=f"b1_{hh}")
                  for hh in range(2)]
            vgw2 = [cpool.tile([P, 5], BF16, tag=f"vgw2_{hh}", name=f"vgw2_{hh}")
                    for hh in range(2)]
            vgb1 = [cpool.tile([P, 1], F32, tag=f"vgb1_{hh}", name=f"vgb1_{hh}")
                    for hh in range(2)]
            for hh in range(2):
                nc.gpsimd.dma_start(w2[hh][:], w2t_d[hh, :, :])
                nc.gpsimd.dma_start(b1[hh][:], b1t_d[hh, :, :])
                nc.gpsimd.dma_start(vgw2[hh][:], vgw2t_d[hh, :, :])
                nc.gpsimd.dma_start(vgb1[hh][:], vgb1t_d[hh, :, :])

            # iota values 0..127 repeated 8x: one-hot build for a whole macro
            iota = cpool.tile([P, 8 * P], BF16, tag="iota")
            nc.gpsimd.iota(iota[:], pattern=[[0, 8], [1, P]], base=0,
                           channel_multiplier=0,
                           allow_small_or_imprecise_dtypes=True)

            relw = cpool.tile([P, ET * 4], F32, tag="relw")
            nc.gpsimd.dma_start(relw[:], relw_d[:, :])
            velg = cpool.tile([P, NT * 18], F32, tag="velg")
            nc.gpsimd.dma_start(velg[:], velg_d[:, :])

            # packed output, one column triple per node tile; single DMA at end
            outbuf = cpool.tile([P, NT * 3], F32, tag="outbuf")

            # ---- PE warmup: ~5us of dummy matmuls off the iota tile so the
            # HAM clock gate reaches 8/8 before the real work lands ----
            ps_warm = ps1_pool.tile([P, 512], F32, tag="ps_mm1", name="warm")
            for _ in range(24):
                nc.tensor.matmul(ps_warm[:], iota[:, 0:P], iota[:, 0:512],
                                 start=True, stop=True)

            # ---- node-path MLP, hoisted: quads of node tiles spread over
            # the edge macros; alpha -> vel_combo + hostc lands in outbuf
            # ahead of the scatter tail ----
            QN = 4                            # node tiles per quad
            n_quad = (NT + QN - 1) // QN
            NODE_EVERY = 8                    # one quad per this many macros
            rhn_tiles = {}

            def node_fetch(q: int):
                lo = QN * q
                nt_n = min(QN, NT - lo)
                rhn = nd_pool.tile([P, QN * 2 * P], BF16, tag="rhn")
                eng = nc.gpsimd
                eng.dma_start(
                    rhn[:, 0:nt_n * 2 * P].rearrange("p (t c) -> p t c",
                                                     t=nt_n),
                    hT[lo:lo + nt_n, :, :].rearrange("t p c -> p t c"))
                rhn_tiles[q] = rhn

            def node_pre(q: int):
                lo = QN * q
                nt_n = min(QN, NT - lo)
                rhn = rhn_tiles.pop(q)
                rhn_v = rhn[:].rearrange("p (t k c) -> p t k c", t=QN, k=2)
                psn = [psw_pool.tile([P, QN * P], F32, tag="ps_w",
                                     name="psn")
                       for _ in range(2)]
                for hh in range(2):
                    for kk in range(2):
                        nc.tensor.matmul(psn[hh][:, 0:nt_n * P],
                                         vgw1[kk][hh][:],
                                         rhn_v[:, 0:nt_n, kk, :],
                                         start=(kk == 0), stop=(kk == 1))
                s1n = [nd_pool.tile([P, QN * P], BF16, tag=f"s1n_{hh}",
                                    name=f"s1n_{hh}")
                       for hh in range(2)]
                for hh in range(2):
                    nc.scalar.activation(s1n[hh][:, 0:nt_n * P],
                                         psn[hh][:, 0:nt_n * P], AF.Silu,
                                         bias=vgb1[hh][:, 0:1], scale=1.0)
                for t in range(nt_n):
                    nt = lo + t
                    psa = psw_pool.tile([P, 8], F32, tag="ps_w")
                    for hh in range(2):
                        nc.tensor.matmul(psa[:, 0:5],
                                         s1n[hh][:, t * P:(t + 1) * P],
                                         vgw2[hh][:],
                                         start=(hh == 0), stop=(hh == 1))
                    # outbuf[:, j] = hostc[:, j] + sum_k alpha[k] * vel[j, k]
                    scratch = sm_pool.tile([P, 15], F32, tag="scratch")
                    vbase = nt * 18
                    velg_v = velg[:, vbase:vbase + 15].rearrange(
                        "p (j k) -> p j k", k=5)
                    nc.vector.tensor_tensor(
                        scratch[:].rearrange("p (j k) -> p j k", k=5),
                        velg_v,
                        psa[:, None, 0:5].broadcast_to([P, 3, 5]),
                        op=OP.mult)
                    acc = sm_pool.tile([P, 3], F32, tag="acc")
                    nc.vector.tensor_reduce(
                        acc[:, :, None],
                        scratch[:].rearrange("p (j k) -> p j k", k=5),
                        axis=mybir.AxisListType.X, op=OP.add)
                    nc.vector.tensor_add(outbuf[:, nt * 3:nt * 3 + 3],
                                         acc[:],
                                         velg[:, vbase + 15:vbase + 18])

            def node_tail(nt: int, ps_sc):
                nc.vector.tensor_add(outbuf[:, nt * 3:nt * 3 + 3],
                                     ps_sc[:, 0:3],
                                     outbuf[:, nt * 3:nt * 3 + 3])

            node_fetch(0)
            node_fetch(1)

            # ---- edge-path macro loop (8 edge tiles per macro) ----
            ps_sc = None
            sup = None
            for m in range(n_mac):
                mq, mr = divmod(m, NODE_EVERY)
                if mr == 0 and mq + 2 < n_quad:
                    node_fetch(mq + 2)
                t0 = m * 8
                G = min(8, ET - t0)          # real edge tiles in this macro
                W = G * P                    # macro width in edges
                si, sm = divmod(m, 2)
                if sm == 0:
                    sup = rhs_pool.tile([P, 4096], FP8, tag="sup", name="sup")
                    eng = nc.sync if si % 2 == 0 else nc.scalar
                    eng.dma_start(sup[:], mijT[si, :, :])
                # [P, 2, 2048]: dim1 = k-half, dim2 = edge within supertile
                sup_v = sup[:].rearrange("p (i e) -> p i e", i=2)
                ps1 = [ps1_pool.tile([P, 1024], F32, tag="ps_mm1", name="ps1")
                       for _ in range(2)]
                s1 = [s1_pool.tile([P, 1024], FP8, tag=f"s1_{hh}",
                                   name=f"s1_{hh}")
                      for hh in range(2)]
                n_ch = (W + MM1_N - 1) // MM1_N
                w1_v = [w1[hh][:].rearrange("p (i m) -> p i m", i=2)
                        for hh in range(2)]
                for hh in range(2):
                    for ch in range(n_ch):
                        cw = min(MM1_N, W - ch * MM1_N)
                        lo = sm * 1024 + ch * MM1_N
                        for kk in range(2):
                            nc.tensor.matmul(
                                ps1[hh][:, ch * MM1_N:ch * MM1_N + cw],
                                w1_v[hh][:, kk, :],
                                sup_v[:, kk, lo:lo + cw],
                                start=(kk == 0), stop=(kk == 1))
                    nc.scalar.activation(s1[hh][:, 0:W], ps1[hh][:, 0:W],
                                         AF.Silu,
                                         bias=b1[hh][:, 0:1],
                                         scale=1.0 / WSCALE)
                    if hh == 0 and mr == 1 and mq < n_quad:
                        node_pre(mq)
                psw = psw_pool.tile([P, 8], F32, tag="ps_w")
                for c in range(G):
                    for hh in range(2):
                        nc.tensor.matmul(psw[:, c:c + 1],
                                         s1[hh][:, c * P:(c + 1) * P],
                                         w2[hh][:],
                                         start=(hh == 0), stop=(hh == 1))

                relw_v = relw.rearrange("p (t f) -> p t f", f=4)
                # msg for all chunks in one op: [P, G, 3] = psw * rel'
                msg = sm_pool.tile([P, 24], BF16, tag="msg")
                nc.vector.tensor_tensor(
                    msg[:, 0:3 * G].rearrange("p (c f) -> p c f", f=3),
                    psw[:, 0:G, None].broadcast_to([P, G, 3]),
                    relw_v[:, t0:t0 + G, 0:3],
                    op=OP.mult)
                # one-hot for all chunks in one op: [P, G*128]
                oh = oh_pool.tile([P, 8 * P], FP8, tag="oh")
                nc.vector.tensor_tensor(
                    oh[:, 0:W].rearrange("p (c f) -> p c f", f=P),
                    iota[:, 0:W].rearrange("p (c f) -> p c f", f=P),
                    relw_v[:, t0:t0 + G, 3:4].broadcast_to([P, G, P]),
                    op=OP.is_equal)

                for c in range(G):
                    t = t0 + c
                    nt, j = divmod(t, K)
                    if j == 0:
                        ps_sc = pssc_pool.tile([P, 3], F32, tag="ps_sc")
                    nc.tensor.matmul(ps_sc[:], oh[:, c * P:(c + 1) * P],
                                     msg[:, 3 * c:3 * c + 3],
                                     start=(j == 0), stop=(j == K - 1))
                    if j == K - 1:
                        node_tail(nt, ps_sc)

            nc.sync.dma_start(out_d[:, :], outbuf[:])

    _split_excess_waits(nc)
    return nc


def _preprocess(inputs: dict):
    """Shard + lay out all per-core device inputs. Returns (in_maps, K)."""
    h = np.asarray(inputs["h"], np.float32)
    m_ij = np.asarray(inputs["m_ij"], np.float32)
    x = np.asarray(inputs["x"], np.float32)
    vel_all = np.asarray(inputs["vel_all"], np.float32)
    ei = np.asarray(inputs["edge_index"])
    src = ei[0].astype(np.int64)
    dst = ei[1].astype(np.int64)

    counts = np.bincount(dst, minlength=N_NODES).astype(np.float32)
    invc = (1.0 / np.maximum(counts, 1.0)).astype(np.float32)

    order = np.argsort(dst, kind="stable")
    dst_s = dst[order]
    src_s = src[order]
    g = dst_s // P                       # global 128-node group, 0..391
    n_groups = N_PAD // P                # 392
    cg = np.bincount(g, minlength=n_groups)
    K = max(1, int(-(-cg.max() // P)))   # ceil(max group)/128
    ET = NT * K

    gstart = np.zeros(n_groups, np.int64)
    gstart[1:] = np.cumsum(cg)[:-1]
    within = np.arange(N_EDGES, dtype=np.int64) - gstart[g]
    slot = g * (K * P) + within          # slot in global [392, K*128] layout

    rel = x[src_s] - x[dst_s]            # [E, 3] in dst-sorted order

    Sg = n_groups * K * P
    colidx = np.full(Sg, -1.0, np.float32)
    colidx[slot] = (dst_s % P).astype(np.float32)
    # rel' folds the scatter mean (invc) and the x16 prescale of W2
    relp = np.zeros((Sg, 3), np.float32)
    relp[slot] = rel * (invc[dst_s] / WSCALE)[:, None]
    mijp = np.zeros((Sg, H), FP8_NP)
    mijp[slot] = m_ij[order].astype(FP8_NP)

    # weights (shared by all cores)
    w1 = np.asarray(inputs["ew_W1"], np.float32)
    b1 = np.asarray(inputs["ew_b1"], np.float32)
    w2 = np.asarray(inputs["ew_W2"], np.float32)
    b2 = float(np.asarray(inputs["ew_b2"], np.float32)[0])
    vgw1 = np.asarray(inputs["vg_W1"], np.float32)
    vgb1 = np.asarray(inputs["vg_b1"], np.float32)
    vgw2 = np.asarray(inputs["vg_W2"], np.float32)
    vgb2 = np.asarray(inputs["vg_b2"], np.float32)

    # host-side constant term per node:
    #   b2 * invc * segsum(rel)  +  sum_k vgb2_k * vel_all[:, k, :]
    g2 = np.stack([np.bincount(dst, weights=rel_c, minlength=N_NODES)
                   for rel_c in (x[src, 0] - x[dst, 0],
                                 x[src, 1] - x[dst, 1],
                                 x[src, 2] - x[dst, 2])], axis=1)
    hostc = b2 * invc[:, None] * g2.astype(np.float32)
    hostc += np.einsum("k,nkj->nj", vgb2, vel_all).astype(np.float32)

    # w1dr[hh][p, i, m] = 16*W1[i*128+p, hh*128+m]
    w1dr = np.ascontiguousarray(
        (WSCALE * w1).reshape(2, P, 2, P).transpose(2, 1, 0, 3)
    ).astype(FP8_NP).reshape(2, P, 2 * P)
    w2t = (WSCALE * w2).reshape(2, P, 1).astype(FP8_NP).copy()
    b1t = b1.reshape(2, P, 1).copy()
    vgw1b = vgw1.reshape(2, P, 2, P).transpose(0, 2, 1, 3).astype(
        BF16_NP).copy()
    vgw2t = vgw2.reshape(2, P, 5).astype(BF16_NP).copy()
    vgb1t = vgb1.reshape(2, P, 1).copy()

    # padded node tensors
    hp = np.zeros((N_PAD, H), np.float32)
    hp[:N_NODES] = h
    velp = np.zeros((N_PAD, 5, 3), np.float32)
    velp[:N_NODES] = vel_all
    hostp = np.zeros((N_PAD, 3), np.float32)
    hostp[:N_NODES] = hostc

    mijp = mijp.reshape(N_CORES, ET * P, H)
    relp = relp.reshape(N_CORES, ET, P, 3)
    colidx = colidx.reshape(N_CORES, ET, P)

    n_mac = (ET + 7) // 8
    n_sup = (n_mac + 1) // 2
    in_maps = []
    for k in range(N_CORES):
        # mijT supertiles: [n_sup, 128, 4096] fp8 DoubleRow layout:
        # [p, i*2048 + e] = m[sup_base + e, i*128 + p]
        full = np.zeros((n_sup * 2048, 2, P), FP8_NP)
        full[:ET * P] = mijp[k].reshape(ET * P, 2, P)
        mijT = np.ascontiguousarray(
            full.reshape(n_sup, 2048, 2, P).transpose(0, 3, 2, 1)
        ).reshape(n_sup, P, 4096)

        rw = np.empty((P, ET, 4), np.float32)
        rw[:, :, 0:3] = relp[k].transpose(1, 0, 2)
        rw[:, :, 3] = colidx[k].T
        relw = np.ascontiguousarray(rw).reshape(P, ET * 4)

        hk = hp[k * NPC:(k + 1) * NPC].reshape(NT, P, H)
        hTk = np.ascontiguousarray(
            hk.transpose(0, 2, 1).reshape(NT, 2, P, P).transpose(0, 2, 1, 3)
        ).reshape(NT, P, 2 * P).astype(BF16_NP)

        # velg cols per node tile: [comp j, gate k] at 5*j+k, hostc at 15:18
        vg = np.empty((P, NT, 18), np.float32)
        vg[:, :, 0:15] = (velp[k * NPC:(k + 1) * NPC]
                          .reshape(NT, P, 5, 3).transpose(1, 0, 3, 2)
                          .reshape(P, NT, 15))
        vg[:, :, 15:18] = (hostp[k * NPC:(k + 1) * NPC]
                           .reshape(NT, P, 3).transpose(1, 0, 2))
        velg = np.ascontiguousarray(vg).reshape(P, NT * 18)

        in_maps.append({
            "mijT": mijT,
            "relw": relw,
            "hT": hTk,
            "velg": velg,
            "w1dr": w1dr,
            "w2t": w2t,
            "b1t": b1t,
            "vgw1b": vgw1b,
            "vgw2t": vgw2t,
            "vgb1t": vgb1t,
        })
    return in_maps, K


def unpack_out(arr: np.ndarray) -> np.ndarray:
    """[128, NT*3] packed per-core output -> [NPC, 3]."""
    return arr.reshape(P, NT, 3).transpose(1, 0, 2).reshape(NPC, 3)


def kernel(**inputs) -> np.ndarray:
    in_maps, K = _preprocess(inputs)
    nc = _build_program(K)
    res = run_bass_kernel_spmd(nc, in_maps, list(range(N_CORES)))
    parts = [unpack_out(res.results[k]["out"]) for k in range(N_CORES)]
    return np.concatenate(parts, axis=0)[:N_NODES].astype(np.float32)
